# revision 10
# baseline (speedup 1.0000x reference)
"""Trainium2 Bass kernel for nn_CustomLoss_35940286333129.

loss[b] = mean|pred-target| (mae, scalar)
        + mean(min_n cdist[b,n,m]) + mean(min_b cdist[b,n,m])  (chamfer, scalar)
        + mean|sort(pred[b].ravel()) - sort(target[b].ravel())|  (emd, per-b)

Sharding: data-parallel over batch B=32 across 8 NeuronCores (4 samples each).

Per-core device kernel (per local sample b, P=pred[b], T=target[b] [1024,128]):
  - PSUM[m, n] = -2*T[m].P[n] + pn[n]  via two accumulating fp16 matmuls:
    stationary -2*T^t tile, then an all-ones stationary over sq16 = PhT*PhT
    (sum_d PhT[d,n]^2 = pn[n] broadcast to every partition) -- the pred-norm
    row never has to be materialized.
  - One fused custom DVE op per [128,1025] tile consumes the PSUM:
        z    = psum + tn[m]          (per-partition scalar)
        out  = where(z < 1e30, min(z, acc), running_min(z))  -> acc (fp16)
    The PSUM pad column 1024 is pre-set to 3e38, so column 1024 of `out`
    captures min_n d2 (the chamfer axis=1 ingredient) while columns 0..1023
    update the running min over local b (the chamfer axis=0 ingredient).
  - tn via one ACT Square pass + DVE innermost-axis reduce; mae from the
    fp16 transposed operands (GpSimd ops + one ACT |.|-accumulate).
  - DMA is split across both HWDGE rings (sync + scalar) so the crossbar
    transposes never queue behind bulk loads; chamfer0 tiles stream out
    during the last sample's compute.
Host: means, cross-core elementwise min + sqrt for chamfer, and the exact
per-sample EMD via np.sort (sort is unsupported on trn2; EMD is 0.015% of
the output value).
"""

import numpy as np

B, N, D = 32, 1024, 128
NCORES = 8
BL = B // NCORES          # 4 local samples per core
NT = N // 128             # 8 row tiles
NPAD = N + 1              # g tile free size (1 scratch col for the scan)

_CACHE = {}


def _register_op():
    from concourse import dve_ops
    from concourse.dve_ops import DveOp, OPS, DveOpSpec
    from concourse.dve_spec import (Spec, Src0, Src1, C0, C1, C2, scan, minn,
                                    select, lower, AluOp)

    for op in OPS:
        if op.name == "MINACC_CH":
            return op

    z = Src0 + C0
    r = scan(AluOp.MIN, z, init=C2)
    body = select(z < C1, minn(z, Src1), r)

    def ref(in0, in1, s0, s1, imm2):
        zz = in0 + s0
        rr = np.minimum.accumulate(np.minimum(zz, imm2), axis=-1)
        return np.where(zz < s1, np.minimum(zz, in1), rr)

    spec = Spec(body=body, reference=ref)
    shas = {}
    for ver in ("v3", "v4"):
        tmp = DveOpSpec(name="MINACC_CH", opcode=0, uops=lower(spec, ver=ver),
                        rd1_en=True)
        shas[ver] = tmp.sha(ver)
    op = DveOp("MINACC_CH", spec, subdim=False, uops_sha=shas)
    OPS.append(op)
    dve_ops.CUSTOM_DVE_SPECS[op.name] = op.spec
    dve_ops._SUB_OPCODE_FOR_NAME[op.name] = (
        dve_ops._CUSTOM_DVE_ROW_BASE + len(OPS) - 1)
    return op


def _build():
    import concourse.bass as bass
    import concourse.bacc as bacc
    import concourse.tile as tile
    from concourse import mybir

    MINACC = _register_op()

    f32, f16 = mybir.dt.float32, mybir.dt.float16
    AF = mybir.ActivationFunctionType
    AL = mybir.AluOpType

    nc = bacc.Bacc("TRN2", target_bir_lowering=False, debug=False,
                   num_devices=NCORES)
    pred = nc.declare_dram_parameter("pred", [BL, N, D], f32, isOutput=False)
    targ = nc.declare_dram_parameter("target", [BL, N, D], f32, isOutput=False)
    mae_o = nc.declare_dram_parameter("mae_part", [128, BL], f32, isOutput=True)
    ch1_o = nc.declare_dram_parameter("ch1_part", [128, BL * NT], f32,
                                      isOutput=True)
    ch0_o = nc.declare_dram_parameter("ch0_part", [N, N], f16, isOutput=True)

    with tile.TileContext(nc) as tc:
        with (
            tc.tile_pool(name="const", bufs=1) as constp,
            tc.tile_pool(name="nat", bufs=2) as natp,
            tc.tile_pool(name="natT", bufs=2) as natTp,
            tc.tile_pool(name="mm", bufs=2) as mmp,
            tc.tile_pool(name="mmT", bufs=2) as mmTp,
            tc.tile_pool(name="sq16", bufs=2) as sq16p,
            tc.tile_pool(name="small", bufs=3) as smallp,
            tc.tile_pool(name="sq", bufs=3) as sqp,
            tc.tile_pool(name="persist", bufs=1) as perp,
            tc.tile_pool(name="nps", bufs=1, space=bass.MemorySpace.PSUM) as nps,
            tc.tile_pool(name="drt", bufs=2, space=bass.MemorySpace.DRAM) as drt,
        ):
            ones128 = constp.tile([128, 128], f16)
            nc.gpsimd.memset(ones128[:], 1.0)

            acc = perp.tile([128, NT, NPAD], f16, tag="acc")
            nc.gpsimd.memset(acc[:], 60000.0)
            ch1z = perp.tile([128, BL * NT], f32, tag="ch1z")
            mae_t = perp.tile([128, BL], f32, tag="mae")

            gtiles = [nps.tile([128, NPAD], f32, tag=f"g{i}", name=f"g{i}")
                      for i in range(2)]
            for gt in gtiles:
                nc.vector.memset(gt[:, N:NPAD], 3.0e38)

            import contextlib

            for b in range(BL):
                prio = tc.high_priority() if b == 0 else contextlib.nullcontext()
                with prio:
                    # --- loads: T on the sync ring, P on the scalar ring ---
                    natT = natTp.tile([128, NT, 128], f32, tag="natT")
                    nc.sync.dma_start(
                        natT[:], targ[b].rearrange("(t p) d -> p t d", p=128))
                    natP = natp.tile([128, NT, 128], f32, tag="natP")
                    nc.scalar.dma_start(
                        natP[:], pred[b].rearrange("(t p) d -> p t d", p=128))

                    # T chain: cast*-2 -> DRAM bounce -> crossbar transpose
                    Th2T = mmTp.tile([128, N], f16, tag="Th2T")
                    nathT = sqp.tile([128, NT, 128], f16, tag="nathT")
                    nc.scalar.mul(nathT[:], natT[:], -2.0)
                    dtrT = drt.tile([N, 128], f16, tag="dtrT")
                    nc.sync.dma_start(
                        dtrT.rearrange("(t p) d -> p t d", p=128), nathT[:])
                    nc.sync.dma_start_transpose(Th2T[:], dtrT[:])

                    # P chain: cast -> bounce + transpose on the scalar ring
                    PhT = mmp.tile([128, N], f16, tag="PhT")
                    nathP = sqp.tile([128, NT, 128], f16, tag="nathP")
                    nc.scalar.mul(nathP[:], natP[:], 1.0)
                    dtrP = drt.tile([N, 128], f16, tag="dtrP")
                    nc.scalar.dma_start(
                        dtrP.rearrange("(t p) d -> p t d", p=128), nathP[:])
                    nc.scalar.dma_start_transpose(PhT[:], dtrP[:])

                    # pn ingredient: sq16[d, n] = PhT^2; the all-ones bias
                    # matmul turns it into sum_d PhT^2 = pn[n] on every
                    # PSUM partition.
                    sq16 = sq16p.tile([128, N], f16, tag="sq16")
                    if b == 0:
                        nc.vector.tensor_mul(sq16[:], PhT[:], PhT[:])
                    else:
                        nc.gpsimd.tensor_mul(sq16[:], PhT[:], PhT[:])

                    # tn: ACT Square + accumulate per 128-col tile (fp32)
                    tncol = smallp.tile([128, NT], f32, tag="tncol")
                    sqsT = sqp.tile([128, NT, 128], f16, tag="sqsT")
                    for t in range(NT):
                        nc.scalar.activation(
                            out=sqsT[:, t, :], in_=natT[:, t, :],
                            func=AF.Square, accum_out=tncol[:, t:t + 1])

                for mt in range(NT):
                    g = gtiles[mt % 2]
                    for c in range(2):
                        nc.tensor.matmul(
                            g[:, c * 512:(c + 1) * 512],
                            Th2T[:, mt * 128:(mt + 1) * 128],
                            PhT[:, c * 512:(c + 1) * 512],
                            start=True, stop=False)
                        nc.tensor.matmul(
                            g[:, c * 512:(c + 1) * 512], ones128[:],
                            sq16[:, c * 512:(c + 1) * 512],
                            start=False, stop=True)
                    nc.vector._custom_dve(
                        MINACC, out=acc[:, mt, :], in0=g[:],
                        in1=acc[:, mt, :], s0=tncol[:, mt:mt + 1],
                        s1=1.0e30, imm2=3.0e38)
                    if b == BL - 1:
                        # acc[mt] is final: stream it out under the remaining
                        # compute instead of serially at the end.
                        nc.sync.dma_start(
                            ch0_o[mt * 128:(mt + 1) * 128, :], acc[:, mt, 0:N])
                # harvest this b's min_n d2 (scratch col) before the next b
                nc.vector.tensor_copy(
                    ch1z[:, b * NT:(b + 1) * NT], acc[:, :, N])

                # --- mae off the critical path: 2(P-T) = 2*PhT + Th2T from
                # the fp16 transposed operands; host divides the sum by 2 ---
                p2 = sqp.tile([128, N], f16, tag="p2")
                nc.gpsimd.tensor_add(p2[:], PhT[:], PhT[:])
                diff = sqp.tile([128, N], f16, tag="diff")
                nc.gpsimd.tensor_add(diff[:], p2[:], Th2T[:])
                absx = sqp.tile([128, N], f16, tag="absx")
                nc.scalar.activation(
                    out=absx[:], in_=diff[:], func=AF.Abs,
                    accum_out=mae_t[:, b:b + 1])

            nc.scalar.sqrt(ch1z[:], ch1z[:])
            nc.sync.dma_start(ch1_o[:], ch1z[:])
            nc.sync.dma_start(mae_o[:], mae_t[:])

    nc.compile()
    return nc


def _get_nc():
    if "nc" not in _CACHE:
        _CACHE["nc"] = _build()
    return _CACHE["nc"]


def run_device(pred, target, trace=False, **kw):
    from concourse.bass_utils import run_bass_kernel_spmd

    nc = _get_nc()
    ins = []
    for i in range(NCORES):
        sl = slice(i * BL, (i + 1) * BL)
        ins.append({
            "pred": np.ascontiguousarray(pred[sl], dtype=np.float32),
            "target": np.ascontiguousarray(target[sl], dtype=np.float32),
        })
    return run_bass_kernel_spmd(nc, ins, list(range(NCORES)), trace=trace, **kw)


def kernel(pred, target):
    pred = np.asarray(pred, dtype=np.float32)
    target = np.asarray(target, dtype=np.float32)
    res = run_device(pred, target)
    rs = res.results

    mae = np.sum([r["mae_part"].astype(np.float64).sum() for r in rs])
    mae /= float(2 * B * N * D)  # device accumulates |2(P-T)|

    ch1 = np.mean([r["ch1_part"].astype(np.float64).mean() for r in rs])

    d0 = rs[0]["ch0_part"].astype(np.float32)
    for r in rs[1:]:
        d0 = np.minimum(d0, r["ch0_part"].astype(np.float32))
    ch0 = np.sqrt(d0.astype(np.float64)).mean()

    p = np.sort(pred.reshape(B, -1), axis=1)
    g = np.sort(target.reshape(B, -1), axis=1)
    emd = np.abs(p - g).mean(axis=1, dtype=np.float64)

    return (mae + ch0 + ch1 + emd).astype(np.float32)
